# revision 1
# baseline (speedup 1.0000x reference)
"""BERT self-attention (B=8, S=1024, HIDDEN=1024, 16 heads x 64) on 8 TRN2 NeuronCores.

Sharding: batch-parallel — core b computes the full attention for batch b.
Per-core program (all matmuls bf16, fp32 PSUM accumulation):
  - inputs arrive host-pre-transposed: xT = x[b].T, wT = W.T (bf16)
  - QT[o,s], KT[o,s] projections (o on partitions -> per-partition bias via
    tensor_scalar_add; 1/sqrt(64) folded into Wq/bq on host)
  - V[s,o] projection stored interleaved with an extra exp(mask) column per
    head ("V_aug": 16 groups x (64 V cols + 1 e col))
  - per head: S^T[k,q] = K @ Q^T (contraction d=64; two heads packed into the
    128 partition rows -> concurrent PE row-tiles), exp on ScalarE (no max
    subtraction needed: scores ~ N(0,1)), P0^T bf16
  - out^T[d,q] (+ denominator row from the e column) = matmul with
    lhsT = V_aug tile, rhs = P0^T; normalize by broadcasted 1/denom
  - host transposes outT back to [S, HIDDEN]
"""

import numpy as np
import ml_dtypes

BF16 = ml_dtypes.bfloat16

B = 8
S = 1024
HID = 1024
H = 16
D = 64
P = 128
NT = HID // P  # 8 tiles of 128
CS = 512       # free-dim chunk (PSUM bank)
CH = S // CS   # 2
GW = D + 1     # V_aug group width (64 V cols + e col)

_CACHE = {}


def build_nc(mask_zero=True, use_gpsimd_bcast=True, order="v5", p0_bufs=28,
             psS_bufs=2, n_copies=1, upto="full", pv_stage=False,
             qk_bufs=2, outp_bufs=4, psA_bufs=2, psV_bufs=2, dma_v2=False):
    """Build + compile the per-core Bass program (same NEFF on all 8 cores)."""
    key = (mask_zero, use_gpsimd_bcast, order, p0_bufs, psS_bufs, n_copies, upto, pv_stage, qk_bufs, outp_bufs, psA_bufs, psV_bufs, dma_v2)
    if key in _CACHE:
        return _CACHE[key]

    import concourse.bacc as bacc
    import concourse.mybir as mybir
    import concourse.tile as tile
    from contextlib import ExitStack

    dt = mybir.dt
    f32 = dt.float32
    bf16 = dt.bfloat16
    EXP = mybir.ActivationFunctionType.Exp
    ADD = mybir.AluOpType.add

    nc = bacc.Bacc("TRN2", target_bir_lowering=False, debug=False, num_devices=B)

    xT_d = nc.dram_tensor("xT", [HID, S], bf16, kind="ExternalInput")
    wqT_d = nc.dram_tensor("wqT", [HID, HID], bf16, kind="ExternalInput")
    wkT_d = nc.dram_tensor("wkT", [HID, HID], bf16, kind="ExternalInput")
    wvT_d = nc.dram_tensor("wvT", [HID, HID], bf16, kind="ExternalInput")
    bq_d = nc.dram_tensor("bq", [P, NT], f32, kind="ExternalInput")
    bk_d = nc.dram_tensor("bk", [P, NT], f32, kind="ExternalInput")
    bvb_d = nc.dram_tensor("bvb", [P, HID], f32, kind="ExternalInput")
    er_d = nc.dram_tensor("er", [P, NT * H], bf16, kind="ExternalInput")
    ef_d = None
    if not mask_zero:
        ef_d = nc.dram_tensor("ef", [P, NT], f32, kind="ExternalInput")
    outT_d = nc.dram_tensor("outT", [HID, S], f32, kind="ExternalOutput")

    with tile.TileContext(nc) as tc:
        with ExitStack() as ctx:
            const = ctx.enter_context(tc.tile_pool(name="const", bufs=1))
            psA = ctx.enter_context(tc.tile_pool(name="psA", bufs=psA_bufs, space="PSUM"))
            psS = ctx.enter_context(
                tc.tile_pool(name="psS", bufs=psS_bufs, space="PSUM"))
            psV = ctx.enter_context(tc.tile_pool(name="psV", bufs=psV_bufs, space="PSUM"))
            qk = ctx.enter_context(tc.tile_pool(name="qk", bufs=qk_bufs))
            p0p = ctx.enter_context(tc.tile_pool(name="p0p", bufs=p0_bufs))
            outp = ctx.enter_context(tc.tile_pool(name="outp", bufs=outp_bufs))

            xT = [const.tile([P, S], bf16, tag=f"xT{t}", name=f"xT{t}")
                  for t in range(NT)]
            wq = [const.tile([P, HID], bf16, tag=f"wq{t}", name=f"wq{t}")
                  for t in range(NT)]
            wk = [const.tile([P, HID], bf16, tag=f"wk{t}", name=f"wk{t}")
                  for t in range(NT)]
            wv = [const.tile([P, HID], bf16, tag=f"wv{t}", name=f"wv{t}")
                  for t in range(NT)]
            bq_sb = const.tile([P, NT], f32, tag="bq", name="bq_sb")
            bk_sb = const.tile([P, NT], f32, tag="bk", name="bk_sb")
            bv_sb = const.tile([P, HID], f32, tag="bv", name="bv_sb")
            ef_sb = None
            if not mask_zero:
                ef_sb = const.tile([P, NT], f32, tag="ef", name="ef_sb")
            v_sb = [const.tile([P, H * GW], bf16, tag=f"v{t}", name=f"v{t}")
                    for t in range(NT)]
            er_sb = const.tile([P, NT * H], bf16, tag="er", name="er_sb")

            def dma_phase1():
                # what the QT/KT projections need; dma_v2 defers wk so the
                # first (Q) projection group can start sooner
                if dma_v2:
                    for t in range(NT):
                        sl = slice(t * P, (t + 1) * P)
                        nc.sync.dma_start(xT[t][:], xT_d.ap()[sl, :])
                        nc.sync.dma_start(wq[t][:], wqT_d.ap()[sl, :])
                    nc.sync.dma_start(bq_sb[:], bq_d.ap()[:])
                    nc.sync.dma_start(bk_sb[:], bk_d.ap()[:])
                    for t in range(NT):
                        sl = slice(t * P, (t + 1) * P)
                        nc.sync.dma_start(wk[t][:], wkT_d.ap()[sl, :])
                    return
                for t in range(NT):
                    sl = slice(t * P, (t + 1) * P)
                    nc.sync.dma_start(xT[t][:], xT_d.ap()[sl, :])
                    nc.sync.dma_start(wq[t][:], wqT_d.ap()[sl, :])
                    nc.sync.dma_start(wk[t][:], wkT_d.ap()[sl, :])
                nc.sync.dma_start(bq_sb[:], bq_d.ap()[:])
                nc.sync.dma_start(bk_sb[:], bk_d.ap()[:])

            def dma_phase2():
                # what the V projection needs
                for t in range(NT):
                    sl = slice(t * P, (t + 1) * P)
                    nc.sync.dma_start(wv[t][:], wvT_d.ap()[sl, :])
                nc.sync.dma_start(bv_sb[:], bvb_d.ap()[:])
                if not mask_zero:
                    nc.sync.dma_start(ef_sb[:], ef_d.ap()[:])
                nc.sync.dma_start(er_sb[:], er_d.ap()[:])
                for t in range(NT):
                    # e columns via DVE (element-exact writes; a scattered
                    # 2-byte DMA here raced with the V-projection writes)
                    dst = v_sb[t].rearrange("p (g e) -> p g e", e=GW)[
                        :, :, D : D + 1]
                    src = er_sb[:, t * H : (t + 1) * H].rearrange(
                        "p (g o) -> p g o", o=1)
                    nc.vector.tensor_copy(dst, src)

            def v_group(t, c):
                ps = psA.tile([P, CS], f32, tag="proj", name="proj_ps")
                for i in range(NT):
                    nc.tensor.matmul(
                        ps[:],
                        xT[i][:, t * P : (t + 1) * P],
                        wv[i][:, c * CS : (c + 1) * CS],
                        start=(i == 0),
                        stop=(i == NT - 1),
                    )
                dst = v_sb[t].rearrange("p (g e) -> p g e", e=GW)[
                    :, c * 8 : (c + 1) * 8, 0:D]
                src = ps.rearrange("p (g d) -> p g d", d=D)
                bvv = bv_sb.rearrange("p (g d) -> p g d", d=D)[
                    :, c * 8 : (c + 1) * 8, :]
                nc.vector.tensor_tensor(dst, src, bvv, op=ADD)
                if c == CH - 1 and not mask_zero:
                    vv = v_sb[t].rearrange("p (g e) -> p g e", e=GW)[:, :, 0:D]
                    nc.vector.tensor_scalar_mul(vv, vv, ef_sb[:, t : t + 1])

            def v_proj_groups():
                return iter([(t, c) for t in range(NT) for c in range(CH)])

            def vg_do(vg, k):
                done = 0
                for t, c in vg:
                    v_group(t, c)
                    done += 1
                    if k is not None and done >= k:
                        break

            def v_proj():
                vg_do(v_proj_groups(), None)

            def qk_alloc():
                qt = qk.tile([P, S], bf16, tag="qt", name="qt")
                kt_t = qk.tile([P, S], bf16, tag="kt", name="kt_t")
                return qt, kt_t

            def qk_group(hp, qt, kt_t, which, c):
                wsb, bsb, dst = ((wq, bq_sb, qt), (wk, bk_sb, kt_t))[which]
                ps = psA.tile([P, CS], f32, tag="proj", name="proj_ps")
                for i in range(NT):
                    nc.tensor.matmul(
                        ps[:],
                        wsb[i][:, hp * P : (hp + 1) * P],
                        xT[i][:, c * CS : (c + 1) * CS],
                        start=(i == 0),
                        stop=(i == NT - 1),
                    )
                nc.vector.tensor_scalar_add(
                    dst[:, c * CS : (c + 1) * CS], ps[:], bsb[:, hp : hp + 1])

            def qk_proj(hp):
                qt = qk.tile([P, S], bf16, tag="qt", name="qt")
                kt_t = qk.tile([P, S], bf16, tag="kt", name="kt_t")
                for wsb, bsb, dst in ((wq, bq_sb, qt), (wk, bk_sb, kt_t)):
                    for c in range(CH):
                        ps = psA.tile([P, CS], f32, tag="proj", name="proj_ps")
                        for i in range(NT):
                            nc.tensor.matmul(
                                ps[:],
                                wsb[i][:, hp * P : (hp + 1) * P],
                                xT[i][:, c * CS : (c + 1) * CS],
                                start=(i == 0),
                                stop=(i == NT - 1),
                            )
                        nc.vector.tensor_scalar_add(
                            dst[:, c * CS : (c + 1) * CS], ps[:],
                            bsb[:, hp : hp + 1])
                return qt, kt_t

            def st_unit(qt, kt_t, kt):
                """S^T matmuls + exp for one k-tile of one head pair."""
                out = []
                for ab in range(2):
                    r0 = ab * D
                    stp = psS.tile([P, S], f32, tag="st", name="stp")
                    for c in range(CH):
                        nc.tensor.matmul(
                            stp[:, c * CS : (c + 1) * CS],
                            kt_t[r0 : r0 + D, kt * P : (kt + 1) * P],
                            qt[r0 : r0 + D, c * CS : (c + 1) * CS],
                            start=True,
                            stop=True,
                            tile_position=(r0, 0),
                        )
                    pt = p0p.tile([P, S], bf16, tag="p0", name="p0t")
                    nc.scalar.activation(pt[:], stp[:], EXP)
                    out.append(pt)
                return out

            def pv_unit(hp, p0, ab, c):
                h = 2 * hp + ab
                pv = psV.tile([GW, CS], f32, tag="pv", name="pv_ps")
                for kt in range(NT):
                    nc.tensor.matmul(
                        pv[:],
                        v_sb[kt][:, h * GW : (h + 1) * GW],
                        p0[ab][kt][:, c * CS : (c + 1) * CS],
                        start=(kt == 0),
                        stop=(kt == NT - 1),
                    )
                if pv_stage:
                    # one DVE copy releases the PSUM slot immediately; the
                    # recip/broadcast/mul tail then runs off the PE critical
                    # path against the SBUF copy
                    pvs = outp.tile([GW, CS], f32, tag="pvs", name="pvs",
                                    bufs=6)
                    nc.vector.tensor_copy(pvs[:], pv[:])
                    pvsrc = pvs
                else:
                    pvsrc = pv
                rc = outp.tile([1, CS], f32, tag="rc", name="rc")
                nc.vector.reciprocal(rc[:], pvsrc[D:GW, :])
                ob = outp.tile([D, CS], f32, tag="ob", name="ob")
                if use_gpsimd_bcast:
                    bc = outp.tile([D, CS], f32, tag="bc", name="bc")
                    nc.gpsimd.partition_broadcast(bc[:], rc[:], channels=D)
                    nc.vector.tensor_mul(ob[:], pvsrc[0:D, :], bc[:])
                else:
                    nc.vector.tensor_mul(
                        ob[:], pvsrc[0:D, :], rc[:].partition_broadcast(D))
                nc.sync.dma_start(
                    outT_d.ap()[h * D : (h + 1) * D, c * CS : (c + 1) * CS],
                    ob[:])

            def pv_head(hp, p0, ab):
                h = 2 * hp + ab
                pvs = [psV.tile([GW, CS], f32, tag="pv", name="pv_ps")
                       for _ in range(CH)]
                for kt in range(NT):
                    for c in range(CH):
                        nc.tensor.matmul(
                            pvs[c][:],
                            v_sb[kt][:, h * GW : (h + 1) * GW],
                            p0[ab][kt][:, c * CS : (c + 1) * CS],
                            start=(kt == 0),
                            stop=(kt == NT - 1),
                        )
                for c in range(CH):
                    pv = pvs[c]
                    rc = outp.tile([1, CS], f32, tag="rc", name="rc")
                    nc.vector.reciprocal(rc[:], pv[D:GW, :])
                    ob = outp.tile([D, CS], f32, tag="ob", name="ob")
                    bc = outp.tile([D, CS], f32, tag="bc", name="bc")
                    nc.gpsimd.partition_broadcast(bc[:], rc[:], channels=D)
                    nc.vector.tensor_mul(ob[:], pv[0:D, :], bc[:])
                    nc.sync.dma_start(
                        outT_d.ap()[h * D : (h + 1) * D, c * CS : (c + 1) * CS],
                        ob[:])

            def st_all(qt, kt_t):
                p0 = [[None] * NT for _ in range(2)]
                for kt in range(NT):
                    a, b_ = st_unit(qt, kt_t, kt)
                    p0[0][kt], p0[1][kt] = a, b_
                return p0

            def pv_all(hp, p0):
                for ab in range(2):
                    for c in range(CH):
                        pv_unit(hp, p0, ab, c)

            def body():
                if upto == "proj":
                    dma_phase1()
                    dma_phase2()
                    v_proj()
                    for hp in range(NT):
                        qk_proj(hp)
                    return
                if upto == "st":
                    dma_phase1()
                    dma_phase2()
                    v_proj()
                    for hp in range(NT):
                        qt, kt_t = qk_proj(hp)
                        st_all(qt, kt_t)
                    return
                if upto == "st_probe":
                    # 128 packed MM pairs (A rows 0-63 / B rows 64-127), no
                    # exp: discriminates concurrent (≈55us) vs serial (≈109us)
                    dma_phase1()
                    qt, kt_t = qk_proj(0)
                    stp_A = psS.tile([P, S], f32, tag="st", name="stpA")
                    stp_B = psS.tile([P, S], f32, tag="st", name="stpB")
                    for i in range(128):
                        c = i % 2
                        for ab, stp in ((0, stp_A), (1, stp_B)):
                            r0 = ab * D
                            nc.tensor.matmul(
                                stp[:, c * CS : (c + 1) * CS],
                                kt_t[r0 : r0 + D, 0:P],
                                qt[r0 : r0 + D, c * CS : (c + 1) * CS],
                                start=True, stop=True, tile_position=(r0, 0))
                    return
                if upto == "mm_probe":
                    # 256 plain full-array MMs (K=128): baseline issue rate
                    dma_phase1()
                    qt, kt_t = qk_proj(0)
                    stp_A = psS.tile([P, S], f32, tag="st", name="stpA")
                    for i in range(256):
                        c = i % 2
                        nc.tensor.matmul(
                            stp_A[:, c * CS : (c + 1) * CS],
                            kt_t[:, 0:P],
                            qt[:, c * CS : (c + 1) * CS],
                            start=True, stop=True)
                    return
                if upto == "exp_probe":
                    dma_phase1()
                    qt, kt_t = qk_proj(0)
                    stp = psS.tile([P, S], f32, tag="st", name="stp")
                    for c in range(CH):
                        nc.tensor.matmul(
                            stp[:, c * CS : (c + 1) * CS],
                            kt_t[0:D, 0:P],
                            qt[0:D, c * CS : (c + 1) * CS],
                            start=True, stop=True, tile_position=(0, 0))
                    for i in range(128):
                        pt = p0p.tile([P, S], bf16, tag="p0", name="p0t")
                        nc.scalar.activation(pt[:], stp[:], EXP)
                    return
                if upto == "pv_only":
                    # PV fed by a CONSTANT p0 (er_sb-backed fake): isolates
                    # PV+normalize cost (no S^T/exp). Reuses v tiles as fake p0.
                    dma_phase1()
                    dma_phase2()
                    v_proj()
                    fake = [[v_sb[kt][:, 0:S] for kt in range(NT)] for _ in range(2)]
                    for hp in range(NT):
                        pv_all(hp, fake)
                    return
                if order == "v1":
                    dma_phase1()
                    dma_phase2()
                    v_proj()
                    for hp in range(NT):
                        qt, kt_t = qk_proj(hp)
                        p0 = st_all(qt, kt_t)
                        pv_all(hp, p0)
                elif order == "v2":
                    dma_phase1()
                    dma_phase2()
                    qt, kt_t = qk_proj(0)
                    p0 = st_all(qt, kt_t)
                    v_proj()
                    pv_all(0, p0)
                    for hp in range(1, NT):
                        qt, kt_t = qk_proj(hp)
                        p0 = st_all(qt, kt_t)
                        pv_all(hp, p0)
                elif order == "v6":
                    # per stage hp: S^T/exp of pair hp (qt/kt built last
                    # stage) with pv(hp-1) units and qk-proj(hp+1) groups
                    # interleaved, so PE always has fill work during the
                    # exp-bound stretches
                    dma_phase1()
                    dma_phase2()
                    qt_c, kt_c = qk_alloc()
                    for w_ in range(2):
                        for c_ in range(CH):
                            qk_group(0, qt_c, kt_c, w_, c_)
                    # stage 0: st(0) + v_proj + proj(1)
                    qt_n, kt_n = qk_alloc()
                    vg = v_proj_groups()
                    sched4 = [(0, 0), (0, 1), (1, 0), (1, 1)]
                    p0_prev = [[None] * NT for _ in range(2)]
                    for kt in range(NT):
                        a, b_ = st_unit(qt_c, kt_c, kt)
                        p0_prev[0][kt], p0_prev[1][kt] = a, b_
                        vg_do(vg, 2)
                        if kt % 2 == 0:
                            w_, c_ = sched4[kt // 2]
                            qk_group(1, qt_n, kt_n, w_, c_)
                    vg_do(vg, None)
                    qt_c, kt_c = qt_n, kt_n
                    # stages 1..7
                    for hp in range(1, NT):
                        if hp < NT - 1:
                            qt_n, kt_n = qk_alloc()
                        p0 = [[None] * NT for _ in range(2)]
                        for kt in range(NT):
                            a, b_ = st_unit(qt_c, kt_c, kt)
                            p0[0][kt], p0[1][kt] = a, b_
                            if kt % 4 == 1:
                                pv_head(hp - 1, p0_prev, kt // 4)
                            elif kt % 2 == 0 and hp < NT - 1:
                                w_, c_ = sched4[kt // 2]
                                qk_group(hp + 1, qt_n, kt_n, w_, c_)
                        p0_prev = p0
                        if hp < NT - 1:
                            qt_c, kt_c = qt_n, kt_n
                    pv_head(NT - 1, p0_prev, 0)
                    pv_head(NT - 1, p0_prev, 1)
                elif order == "v5":
                    # per stage hp: S^T/exp of pair hp (qt/kt built last
                    # stage) with pv(hp-1) units and qk-proj(hp+1) groups
                    # interleaved, so PE always has fill work during the
                    # exp-bound stretches
                    dma_phase1()
                    dma_phase2()
                    qt_c, kt_c = qk_alloc()
                    for w_ in range(2):
                        for c_ in range(CH):
                            qk_group(0, qt_c, kt_c, w_, c_)
                    # stage 0: st(0) + v_proj + proj(1)
                    qt_n, kt_n = qk_alloc()
                    vg = v_proj_groups()
                    sched4 = [(0, 0), (0, 1), (1, 0), (1, 1)]
                    p0_prev = [[None] * NT for _ in range(2)]
                    for kt in range(NT):
                        a, b_ = st_unit(qt_c, kt_c, kt)
                        p0_prev[0][kt], p0_prev[1][kt] = a, b_
                        vg_do(vg, 2)
                        if kt % 2 == 0:
                            w_, c_ = sched4[kt // 2]
                            qk_group(1, qt_n, kt_n, w_, c_)
                    vg_do(vg, None)
                    qt_c, kt_c = qt_n, kt_n
                    # stages 1..7
                    for hp in range(1, NT):
                        if hp < NT - 1:
                            qt_n, kt_n = qk_alloc()
                        p0 = [[None] * NT for _ in range(2)]
                        for kt in range(NT):
                            a, b_ = st_unit(qt_c, kt_c, kt)
                            p0[0][kt], p0[1][kt] = a, b_
                            if kt % 2 == 1:
                                ab, c_ = sched4[kt // 2]
                                pv_unit(hp - 1, p0_prev, ab, c_)
                            elif hp < NT - 1:
                                w_, c_ = sched4[kt // 2]
                                qk_group(hp + 1, qt_n, kt_n, w_, c_)
                        p0_prev = p0
                        if hp < NT - 1:
                            qt_c, kt_c = qt_n, kt_n
                    pv_all(NT - 1, p0_prev)
                elif order in ("v3", "v4"):
                    # software-pipelined emission: PV(hp-1) interleaved
                    # between the S^T/exp units of pair hp. v4 additionally
                    # interleaves the V projection into pair 0's S^T loop so
                    # ScalarE gets exp work while PE does the V matmuls.
                    dma_phase1()
                    dma_phase2()
                    qt, kt_t = qk_proj(0)
                    if order == "v3":
                        p0_prev = st_all(qt, kt_t)
                        v_proj()
                    else:
                        vg = v_proj_groups()
                        p0_prev = [[None] * NT for _ in range(2)]
                        for kt in range(NT):
                            a, b_ = st_unit(qt, kt_t, kt)
                            p0_prev[0][kt], p0_prev[1][kt] = a, b_
                            vg_do(vg, 2)
                        vg_do(vg, None)
                    for hp in range(1, NT):
                        qt, kt_t = qk_proj(hp)
                        p0 = [[None] * NT for _ in range(2)]
                        pv_sched = [(0, 0), (0, 1), (1, 0), (1, 1)]
                        for kt in range(NT):
                            a, b_ = st_unit(qt, kt_t, kt)
                            p0[0][kt], p0[1][kt] = a, b_
                            if kt % 2 == 1:
                                ab, c = pv_sched[kt // 2]
                                pv_unit(hp - 1, p0_prev, ab, c)
                        p0_prev = p0
                    pv_all(NT - 1, p0_prev)
                else:
                    raise ValueError(order)

            for _rep in range(n_copies):
                body()

    nc.compile()
    _CACHE[key] = nc
    return nc


def prepare_in_maps(x, attention_mask, Wq, bq, Wk, bk, Wv, bv):
    x = np.asarray(x, np.float32)
    attention_mask = np.asarray(attention_mask, np.float32)
    Wq = np.asarray(Wq, np.float32)
    Wk = np.asarray(Wk, np.float32)
    Wv = np.asarray(Wv, np.float32)
    bq = np.asarray(bq, np.float32)
    bk = np.asarray(bk, np.float32)
    bv = np.asarray(bv, np.float32)

    scale = np.float32(1.0 / np.sqrt(D))
    wqT = np.ascontiguousarray((Wq * scale).T).astype(BF16)
    wkT = np.ascontiguousarray(Wk.T).astype(BF16)
    wvT = np.ascontiguousarray(Wv.T).astype(BF16)
    bqh = np.ascontiguousarray((bq * scale).reshape(NT, P).T)
    bkh = np.ascontiguousarray(bk.reshape(NT, P).T)
    bvbh = np.ascontiguousarray(np.broadcast_to(bv, (P, HID)))

    mask_zero = not np.any(attention_mask)

    in_maps = []
    for b in range(B):
        xT = np.ascontiguousarray(x[b].T).astype(BF16)
        e = np.exp(attention_mask[b, 0, 0, :]).astype(np.float32)
        e2 = e.astype(BF16).reshape(NT, P).T  # [P, NT]
        er = np.ascontiguousarray(
            np.repeat(e2[:, :, None], H, axis=2).reshape(P, NT * H))
        m = dict(xT=xT, wqT=wqT, wkT=wkT, wvT=wvT, bq=bqh, bk=bkh, bvb=bvbh,
                 er=er)
        if not mask_zero:
            m["ef"] = np.ascontiguousarray(e.reshape(NT, P).T)
        in_maps.append(m)
    return in_maps, mask_zero


def kernel(x, attention_mask, Wq, bq, Wk, bk, Wv, bv):
    from concourse.bass_utils import run_bass_kernel_spmd

    in_maps, mask_zero = prepare_in_maps(
        x, attention_mask, Wq, bq, Wk, bk, Wv, bv)
    nc = build_nc(mask_zero=mask_zero)
    res = run_bass_kernel_spmd(nc, in_maps, core_ids=list(range(B)))
    y = np.empty((B, S, HID), np.float32)
    for b in range(B):
        y[b] = res.results[b]["outT"].T
    return y

